# revision 25
# baseline (speedup 1.0000x reference)
# Trainium2 Bass kernel for BaseGumbelGraphNetwork message passing.
#
# Reference computation (B=4, N=512, D=2, H=64, O=2):
#   e1 = relu(cat(x_i, x_j) @ W_n2e.T + b_n2e)        [B,N,N,H]
#   e2 = relu(e1 @ W_e2e.T + b_e2e)                   [B,N,N,H]
#   s  = sum_j adj[i,j] * e2                          [B,N,H]
#   h  = relu(relu(s@W_e2n.T+b)@W_n2n.T+b)
#   out= relu(cat(x,h)@W_o1.T+b) @ W_o2.T + b         [B,N,O]
#
# Key structure: layer 1 factorizes over the (i,j) grid:
#   e1[b,i,j,:] = relu(A[b,i,:] + C[b,j,:] + b1),  A = x@Wi.T, C = x@Wj.T
# so the [B,N,N,2D] concat tensor is never materialized.
#
# Device layout (per core, i-dim sharded 8 ways -> 64 rows/core):
#   * a unit is (batch b, i-pair q): partitions = h stacked for the two i's
#     (2x64), free dim = j (512). Two consecutive q's share one [128,1024]
#     tile. Loop: b outer, q-pair inner, global iteration g = 16*b + t.
#   * per iteration g:
#       2x DVE tensor_scalar  e1 = relu(C.T + (A_i + b1))   fp16
#       2x PE matmul          e2pre = blockdiag(W_e2e.T).T @ e1  (fp16)
#       1x ACT                e2 = relu(e2pre + b2)  PSUM -> SBUF bf16
#       1x DVE tensor_tensor  scr = e2 * adj_bcast   [128,1024] bf16
#       2x PE matmul (fused reduce): h1pre[:, b, q] = sum_j W_e2n_bd.T @ scr
#          via a PSUM output AP repeating 8 columns 64 times -- PSUM's
#          per-element accumulate sums all 512 j-columns in hardware.
#   * CRITICAL SCHEDULING: each engine's queue runs in-order, so the e1
#     tensor_scalars are emitted AHEAD=3 iterations early.  Otherwise the
#     steady state serializes on the cross-engine cycle
#     ACT(g-1) -> TT(g-1) -> TS(g) -> MM(g) -> ACT(g) and every iteration
#     pays the full latency chain (~2.1us) instead of the busiest engine
#     time (~1.4us).  TT lags its ACT by 1, reduce-MMs lag TT by 1.
#   * the tiny output MLP runs once at the end, batched over all 4 b's on
#     [128, 128] tiles (weights in fp16 so the matmuls are cheap).
#   * adj rows are partition-broadcast into SBUF by DMA once (during the
#     b=0 pass) and stay resident for all four batches.

import numpy as np

B, N, D, H, O = 4, 512, 2, 64, 2
NCORES = 8
IB = N // NCORES  # i rows per core = 64
Q = IB // 2       # i pairs per core = 32
T = Q // 2        # q-pair iterations per batch = 16
G = B * T         # total main-loop iterations = 64
AHEAD = 3         # e1 build lead (iterations)

_STATE = {}

# wpack column layout (fp32, 128 partitions): per-partition bias vectors
_WP = {}
_o = 0
for _name, _w in [("b1s", 1), ("b2s", 1), ("be2ns", 1), ("bn2ns", 1),
                  ("bo1s", 1), ("bo2s", 1)]:
    _WP[_name] = (_o, _o + _w)
    _o += _w
WPACK_COLS = _o

# w2pk column layout (fp16, 128 partitions): matmul weights
_W2 = {}
_o = 0
for _name, _w in [("w2bd", 128), ("wn2nbd", 128), ("wo1hbd", 128),
                  ("wo2bd", 4)]:
    _W2[_name] = (_o, _o + _w)
    _o += _w
W2PK_COLS = _o

# xpk column layout (fp32, 4 partitions; per-b block after the fixed part)
XB = 512 + Q + Q + Q  # xT, xtie, xtio, xpair widths per b
XPK_FIX = 128 + 64 + 128  # wjt2, wit, wo1xbd
XPK_COLS = XPK_FIX + B * XB


def _build_nc():
    import concourse.mybir as mybir
    from concourse import bacc
    from concourse.tile import TileContext

    F32 = mybir.dt.float32
    FP16 = mybir.dt.float16   # e1 / W2-matmul path (better weight precision)
    BFL = mybir.dt.bfloat16   # e2 / mask / reduce path
    AL = mybir.AluOpType
    AF = mybir.ActivationFunctionType

    nc = bacc.Bacc("TRN2", target_bir_lowering=False, debug=False,
                   num_devices=NCORES)

    def din(name, shape, dt=F32):
        return nc.dram_tensor(name, list(shape), dt, kind="ExternalInput").ap()

    wpack = din("wpack", (128, WPACK_COLS))
    w2pk = din("w2pk", (128, W2PK_COLS), FP16)
    xpk = din("xpk", (4, XPK_COLS), FP16)
    adjr = din("adjr", (IB, N), BFL)         # this core's adjacency rows
    we2nbd = din("we2nbd", (128, 128), BFL)  # blockdiag(W_e2n.T, W_e2n.T)

    out_d = nc.dram_tensor("out", [B, IB, O], F32, kind="ExternalOutput").ap()
    # out[b, 2q+e, o] <- OUT_sb[(e o), b*Q + q]
    out_re = out_d.rearrange("b (q e) o -> b (e o) q", e=2)

    with TileContext(nc, pool_alloc_mode="queue") as tc:
        with (tc.tile_pool(name="wpool", bufs=1) as wp,
              tc.tile_pool(name="ctsp", bufs=B) as ctsp,
              tc.tile_pool(name="abp", bufs=B) as abp,
              tc.tile_pool(name="adjp", bufs=T) as adjp,
              tc.tile_pool(name="e1p", bufs=AHEAD + 4) as e1p,
              tc.tile_pool(name="e2p", bufs=6) as e2p,
              tc.tile_pool(name="scrp", bufs=6) as scrp,
              tc.tile_pool(name="finp", bufs=2) as finp,
              tc.tile_pool(name="psp", bufs=3, space="PSUM") as psp,
              tc.tile_pool(name="hps", bufs=2, space="PSUM") as hps):

            # ---- packed loads, one per HWDGE ring so they land in
            # parallel.  xpk (which gates the b=0 setup chain) is split so
            # its fixed+b0 slice arrives first; the scalar/vector queues are
            # idle this early so the descriptor-gen time there is free. ----
            xpkt = wp.tile([4, XPK_COLS], FP16, tag="xpk")
            xcut = XPK_FIX + XB
            nc.sync.dma_start(out=xpkt[:, 0:xcut], in_=xpk[:, 0:xcut])
            wpk = wp.tile([128, WPACK_COLS], F32, tag="wpk")
            nc.scalar.dma_start(out=wpk[:], in_=wpack[:])
            w2pkt = wp.tile([128, W2PK_COLS], FP16, tag="w2pk")
            nc.gpsimd.dma_start(out=w2pkt[:], in_=w2pk[:])
            we2nbd_s = wp.tile([128, 128], BFL, tag="we2nbd")
            nc.scalar.dma_start(out=we2nbd_s[:], in_=we2nbd[:])
            nc.gpsimd.dma_start(out=xpkt[:, xcut:XPK_COLS],
                                in_=xpk[:, xcut:XPK_COLS])

            def wslice(name):
                a, bb = _WP[name]
                return wpk[:, a:bb]
            b1s_s, b2s_s = wslice("b1s"), wslice("b2s")
            be2ns_s, bn2ns_s = wslice("be2ns"), wslice("bn2ns")
            bo1s_s = wslice("bo1s")
            bo2s_s = wpk[0:4, _WP["bo2s"][0]:_WP["bo2s"][1]]

            def w2slice(name, rows=128):
                a, bb = _W2[name]
                return w2pkt[0:rows, a:bb]
            w2bd_s = w2slice("w2bd")
            wn2nbd_s, wo1hbd_s = w2slice("wn2nbd"), w2slice("wo1hbd")
            wo2bd_s = w2slice("wo2bd")
            wjt2_s = xpkt[0:2, 0:128]
            wit_s = xpkt[0:2, 128:192]
            wo1xbd_s = xpkt[0:4, 192:XPK_FIX]

            def xslice(b, off, w, rows=2):
                a = XPK_FIX + b * XB + off
                return xpkt[0:rows, a:a + w]

            # fused-reduce accumulators: h1pre 8-sub-columns per (b, q);
            # two PSUM banks, one per batch-pair
            h1ps = [hps.tile([128, 2 * Q * 8], F32, tag="h1ps",
                             name=f"h1ps{i}")
                    for i in range(2)]
            h1v = [t[:].rearrange("p (b q e) -> p b q e", b=2, e=8)
                   for t in h1ps]

            # ---- per-batch setup: CTS (stacked C.T) and ABIAS (A + b1);
            # emitted lazily so batch 0 reaches the main loop ASAP ----
            CTS, AB = [None] * B, [None] * B

            def emit_setup(b):
                ps = psp.tile([128, 1024], F32, tag="ps")
                nc.tensor.matmul(ps[:, 0:512], lhsT=wjt2_s,
                                 rhs=xslice(b, 0, 512), start=True, stop=True)
                nc.tensor.matmul(ps[0:64, 512:512 + Q], lhsT=wit_s,
                                 rhs=xslice(b, 512, Q), start=True, stop=True)
                nc.tensor.matmul(ps[64:128, 512:512 + Q], lhsT=wit_s,
                                 rhs=xslice(b, 512 + Q, Q),
                                 start=True, stop=True)
                cts = ctsp.tile([128, N], FP16, tag="cts")
                nc.scalar.copy(cts[:], ps[:, 0:512])
                ab = abp.tile([128, Q], F32, tag="ab")
                nc.vector.tensor_scalar_add(out=ab[:],
                                            in0=ps[0:128, 512:512 + Q],
                                            scalar1=b1s_s)
                CTS[b], AB[b] = cts, ab

            emit_setup(0)

            # ---- main loop, software pipelined ----
            ADJ = [None] * T
            E1 = [None] * G     # e1 tiles, built AHEAD iterations early
            E2 = [None] * G
            SCR = [None] * G

            def emit_e1(g):
                b, t = divmod(g, T)
                e1m = e1p.tile([128, 1024], FP16, tag="e1", name=f"e1_{g}")
                for k in range(2):
                    q = 2 * t + k
                    nc.vector.tensor_scalar(
                        out=e1m[:, 512 * k:512 * (k + 1)], in0=CTS[b][:],
                        scalar1=AB[b][:, q:q + 1], scalar2=0.0,
                        op0=AL.add, op1=AL.max)
                E1[g] = e1m

            def emit_tt(g):
                b, t = divmod(g, T)
                scrm = scrp.tile([128, 1024], BFL, tag="scr", name=f"scr{g}")
                nc.vector.tensor_tensor(
                    out=scrm[:].rearrange("p (u j) -> p u j", u=2),
                    in0=E2[g][:].rearrange("p (u j) -> p u j", u=2),
                    in1=ADJ[t][:].rearrange("p (u j) -> p u j", u=2),
                    op=AL.mult)
                SCR[g] = scrm

            def emit_red(g):
                b, t = divmod(g, T)
                for k in range(2):
                    q = 2 * t + k
                    sl = h1v[b // 2][:, b % 2, q:q + 1, :]       # [128, 1, 8]
                    nc.tensor.matmul(sl.broadcast_to((128, 64, 8)),
                                     lhsT=we2nbd_s[:],
                                     rhs=SCR[g][:, 512 * k:512 * (k + 1)],
                                     start=True, stop=True)

            for g in range(AHEAD):
                emit_e1(g)

            def load_adj(t):
                adjt = adjp.tile([128, 1024], BFL, tag="adj")
                for r in range(4):
                    # alternate the two HWDGE rings: sync + scalar
                    eng = nc.sync if r % 2 == 0 else nc.scalar
                    eng.dma_start(
                        out=adjt[64 * (r % 2):64 * (r % 2 + 1),
                                 512 * (r // 2):512 * (r // 2 + 1)],
                        in_=adjr[4 * t + r:4 * t + r + 1, :]
                        .partition_broadcast(64))
                ADJ[t] = adjt

            load_adj(0)
            load_adj(1)
            for g in range(G):
                b, t = divmod(g, T)
                if b == 0 and t + 2 < T:
                    load_adj(t + 2)   # prefetch two iterations ahead
                if 1 <= g < B:
                    emit_setup(g)     # batches 1-3 set up during early iters
                if g + AHEAD < G:
                    emit_e1(g + AHEAD)
                psm = psp.tile([128, 1024], F32, tag="ps")
                nc.tensor.matmul(psm[:, 0:512], lhsT=w2bd_s[:],
                                 rhs=E1[g][:, 0:512], start=True, stop=True)
                nc.tensor.matmul(psm[:, 512:1024], lhsT=w2bd_s[:],
                                 rhs=E1[g][:, 512:1024], start=True,
                                 stop=True)
                e2m = e2p.tile([128, 1024], BFL, tag="e2", name=f"e2_{g}")
                nc.scalar.activation(e2m[:], psm[:], AF.Relu, bias=b2s_s)
                E2[g] = e2m
                if g >= 1:
                    emit_tt(g - 1)
                if g >= 2:
                    emit_red(g - 2)
            emit_tt(G - 1)
            emit_red(G - 2)
            emit_red(G - 1)

            # ---- batched output MLP over all 4 b's: [128, B*Q] tiles ----
            BQ = B * Q
            # h1pre = sum over the 8 PSUM sub-columns; cols are b-major
            h1pre = finp.tile([128, BQ], F32, tag="h1pre")
            for p in range(2):
                nc.vector.tensor_reduce(
                    out=h1pre[:, 64 * p:64 * (p + 1)]
                    .rearrange("p (b q) -> p b q", b=2),
                    in_=h1v[p], axis=mybir.AxisListType.X, op=AL.add)
            h1 = finp.tile([128, BQ], FP16, tag="h1")
            nc.scalar.activation(h1[:], h1pre[:], AF.Relu, bias=be2ns_s)

            ps2 = psp.tile([128, 1024], F32, tag="ps")
            nc.tensor.matmul(ps2[:, 0:BQ], lhsT=wn2nbd_s, rhs=h1[:],
                             start=True, stop=True)
            h2 = finp.tile([128, BQ], FP16, tag="h2")
            nc.scalar.activation(h2[:], ps2[:, 0:BQ], AF.Relu, bias=bn2ns_s)

            ps3 = psp.tile([128, 1024], F32, tag="ps")
            # cat(x, h2) @ W_o1.T as two accumulating matmuls per b block
            for b in range(B):
                nc.tensor.matmul(ps3[:, b * Q:(b + 1) * Q], lhsT=wo1hbd_s,
                                 rhs=h2[:, b * Q:(b + 1) * Q],
                                 start=True, stop=False)
                nc.tensor.matmul(ps3[:, b * Q:(b + 1) * Q], lhsT=wo1xbd_s,
                                 rhs=xslice(b, 512 + 2 * Q, Q, rows=4),
                                 start=False, stop=True)
            h3 = finp.tile([128, BQ], FP16, tag="h3")
            nc.scalar.activation(h3[:], ps3[:, 0:BQ], AF.Relu, bias=bo1s_s)

            ps4 = psp.tile([128, 1024], F32, tag="ps")
            nc.tensor.matmul(ps4[0:4, 0:BQ], lhsT=wo2bd_s, rhs=h3[:],
                             start=True, stop=True)
            outs = finp.tile([4, BQ], F32, tag="outs")
            nc.scalar.activation(outs[:], ps4[0:4, 0:BQ], AF.Identity,
                                 bias=bo2s_s)
            for b in range(B):
                eng = nc.sync if b % 2 == 0 else nc.gpsimd
                eng.dma_start(out=out_re[b], in_=outs[:, b * Q:(b + 1) * Q])

    nc.compile()
    return nc


def _get_nc():
    if "nc" not in _STATE:
        _STATE["nc"] = _build_nc()
    return _STATE["nc"]


def _prep_maps(inputs):
    import ml_dtypes
    bfl = ml_dtypes.bfloat16
    fp16 = np.float16
    f32 = np.float32

    x = np.ascontiguousarray(np.asarray(inputs["input"], f32))      # [B,N,D]
    adj = np.ascontiguousarray(np.asarray(inputs["adj"], f32))      # [N,N]
    W_n2e = np.asarray(inputs["W_n2e"], f32)   # [H, 2D]
    b_n2e = np.asarray(inputs["b_n2e"], f32)
    W_e2e = np.asarray(inputs["W_e2e"], f32)
    b_e2e = np.asarray(inputs["b_e2e"], f32)
    W_e2n = np.asarray(inputs["W_e2n"], f32)
    b_e2n = np.asarray(inputs["b_e2n"], f32)
    W_n2n = np.asarray(inputs["W_n2n"], f32)
    b_n2n = np.asarray(inputs["b_n2n"], f32)
    W_o1 = np.asarray(inputs["W_o1"], f32)     # [H, D+H]
    b_o1 = np.asarray(inputs["b_o1"], f32)
    W_o2 = np.asarray(inputs["W_o2"], f32)     # [O, H]
    b_o2 = np.asarray(inputs["b_o2"], f32)

    Wi, Wj = W_n2e[:, :D], W_n2e[:, D:]

    def bd(w):  # blockdiag(w, w)
        r, c = w.shape
        z = np.zeros((2 * r, 2 * c), f32)
        z[:r, :c] = w
        z[r:, c:] = w
        return z

    wpack = np.zeros((128, WPACK_COLS), f32)

    def put(name, val, rows=128):
        a, bb = _WP[name]
        wpack[:rows, a:bb] = val
    put("b1s", np.concatenate([b_n2e, b_n2e]).reshape(128, 1))
    put("b2s", np.concatenate([b_e2e, b_e2e]).reshape(128, 1))
    put("be2ns", np.concatenate([b_e2n, b_e2n]).reshape(128, 1))
    put("bn2ns", np.concatenate([b_n2n, b_n2n]).reshape(128, 1))
    put("bo1s", np.concatenate([b_o1, b_o1]).reshape(128, 1))
    put("bo2s", np.concatenate([b_o2, b_o2]).reshape(4, 1), rows=4)

    w2pack = np.zeros((128, W2PK_COLS), f32)

    def put2(name, val, rows=128):
        a, bb = _W2[name]
        w2pack[:rows, a:bb] = val
    put2("w2bd", bd(W_e2e.T))
    put2("wn2nbd", bd(W_n2n.T))
    put2("wo1hbd", bd(W_o1[:, D:].T))
    put2("wo2bd", bd(W_o2.T))

    maps = []
    for c in range(NCORES):
        sl = slice(c * IB, (c + 1) * IB)
        xc = x[:, sl]                                    # [B, IB, D]
        xpk = np.zeros((4, XPK_COLS), f32)
        xpk[0:2, 0:128] = np.concatenate([Wj.T, Wj.T], axis=1)
        xpk[0:2, 128:192] = Wi.T
        xpk[0:4, 192:XPK_FIX] = bd(W_o1[:, :D].T)
        for b in range(B):
            a = XPK_FIX + b * XB
            xpk[0:2, a:a + 512] = x[b].T
            xpk[0:2, a + 512:a + 512 + Q] = xc[b, 0::2].T
            xpk[0:2, a + 512 + Q:a + 512 + 2 * Q] = xc[b, 1::2].T
            xpk[0:4, a + 512 + 2 * Q:a + 512 + 3 * Q] = \
                xc[b].reshape(Q, 2 * D).T                # rows e*2+d
        m = {
            "wpack": wpack,
            "w2pk": w2pack.astype(fp16),
            "xpk": xpk.astype(fp16),
            "adjr": adj[sl].astype(bfl),
            "we2nbd": bd(W_e2n.T).astype(bfl),
        }
        maps.append({k: np.ascontiguousarray(v) for k, v in m.items()})
    return maps


def run(inputs, trace=False, **kw):
    from concourse.bass_utils import run_bass_kernel_spmd
    nc = _get_nc()
    maps = _prep_maps(inputs)
    res = run_bass_kernel_spmd(nc, maps, list(range(NCORES)), trace=trace, **kw)
    out = np.concatenate([res.results[c]["out"] for c in range(NCORES)], axis=1)
    return np.ascontiguousarray(out, dtype=np.float32), res


def kernel(**inputs):
    out, _ = run(inputs, trace=False)
    return out


# revision 30
# speedup vs baseline: 1.0811x; 1.0811x over previous
# Trainium2 Bass kernel for BaseGumbelGraphNetwork message passing.
#
# Reference computation (B=4, N=512, D=2, H=64, O=2):
#   e1 = relu(cat(x_i, x_j) @ W_n2e.T + b_n2e)        [B,N,N,H]
#   e2 = relu(e1 @ W_e2e.T + b_e2e)                   [B,N,N,H]
#   s  = sum_j adj[i,j] * e2                          [B,N,H]
#   h  = relu(relu(s@W_e2n.T+b)@W_n2n.T+b)
#   out= relu(cat(x,h)@W_o1.T+b) @ W_o2.T + b         [B,N,O]
#
# Key structure: layer 1 factorizes over the (i,j) grid:
#   e1[b,i,j,:] = relu(A[b,i,:] + C[b,j,:] + b1),  A = x@Wi.T, C = x@Wj.T
# so the [B,N,N,2D] concat tensor is never materialized.
#
# Device layout (per core, i-dim sharded 8 ways -> 64 rows/core):
#   * a unit is (batch b, i-pair q): partitions = h stacked for the two i's
#     (2x64), free dim = j (512). Two consecutive q's share one [128,1024]
#     tile. Loop: b outer, q-pair inner, global iteration g = 16*b + t.
#   * per iteration g:
#       2x DVE tensor_scalar  e1 = relu(C.T + (A_i + b1))   fp16
#       2x PE matmul          e2pre = blockdiag(W_e2e.T).T @ e1  (fp16)
#       1x ACT                e2 = relu(e2pre + b2)  PSUM -> SBUF bf16
#       1x DVE tensor_tensor  scr = e2 * adj_bcast   [128,1024] bf16
#       2x PE matmul (fused reduce): h1pre[:, b, q] = sum_j W_e2n_bd.T @ scr
#          via a PSUM output AP repeating 8 columns 64 times -- PSUM's
#          per-element accumulate sums all 512 j-columns in hardware.
#   * CRITICAL SCHEDULING: each engine's queue runs in-order, so the e1
#     tensor_scalars are emitted AHEAD=3 iterations early.  Otherwise the
#     steady state serializes on the cross-engine cycle
#     ACT(g-1) -> TT(g-1) -> TS(g) -> MM(g) -> ACT(g) and every iteration
#     pays the full latency chain (~2.1us) instead of the busiest engine
#     time (~1.4us).  TT lags its ACT by 1, reduce-MMs lag TT by 1.
#   * the tiny output MLP runs once at the end, batched over all 4 b's on
#     [128, 128] tiles (weights in fp16 so the matmuls are cheap).
#   * adj rows are partition-broadcast into SBUF by DMA once (during the
#     b=0 pass) and stay resident for all four batches.

import numpy as np

B, N, D, H, O = 4, 512, 2, 64, 2
NCORES = 8
IB = N // NCORES  # i rows per core = 64
Q = IB // 2       # i pairs per core = 32
T = Q // 2        # q-pair iterations per batch = 16
G = B * T         # total main-loop iterations = 64
AHEAD = 3         # e1 build lead (iterations)

_STATE = {}

# wpack column layout (fp32, 128 partitions): per-partition bias vectors
_WP = {}
_o = 0
for _name, _w in [("b1s", 1), ("b2s", 1), ("be2ns", 1), ("bn2ns", 1),
                  ("bo1s", 1), ("bo2s", 1)]:
    _WP[_name] = (_o, _o + _w)
    _o += _w
WPACK_COLS = _o

# w2pk column layout (fp16, 128 partitions): matmul weights
_W2 = {}
_o = 0
for _name, _w in [("w2bd", 128), ("wn2nbd", 128), ("wo1hbd", 128),
                  ("wo2bd", 4)]:
    _W2[_name] = (_o, _o + _w)
    _o += _w
W2PK_COLS = _o

# xpk column layout (fp32, 4 partitions; per-b block after the fixed part)
XB = 512 + Q + Q + Q  # xT, xtie, xtio, xpair widths per b
XPK_FIX = 128 + 64 + 128  # wjt2, wit, wo1xbd
XPK_COLS = XPK_FIX + B * XB


def _build_nc():
    import concourse.mybir as mybir
    from concourse import bacc
    from concourse.tile import TileContext

    F32 = mybir.dt.float32
    FP16 = mybir.dt.float16   # e1 / W2-matmul path (better weight precision)
    BFL = mybir.dt.bfloat16   # e2 / mask / reduce path
    AL = mybir.AluOpType
    AF = mybir.ActivationFunctionType

    nc = bacc.Bacc("TRN2", target_bir_lowering=False, debug=False,
                   num_devices=NCORES)

    def din(name, shape, dt=F32):
        return nc.dram_tensor(name, list(shape), dt, kind="ExternalInput").ap()

    wpack = din("wpack", (128, WPACK_COLS))
    w2pk = din("w2pk", (128, W2PK_COLS), FP16)
    xpk = din("xpk", (4, XPK_COLS), FP16)
    adjr = din("adjr", (IB, N), BFL)         # this core's adjacency rows
    we2nbd = din("we2nbd", (128, 128), BFL)  # blockdiag(W_e2n.T, W_e2n.T)

    out_d = nc.dram_tensor("out", [B, IB, O], F32, kind="ExternalOutput").ap()
    # out[b, 2q+e, o] <- OUT_sb[(e o), b*Q + q]
    out_re = out_d.rearrange("b (q e) o -> b (e o) q", e=2)

    with TileContext(nc, pool_alloc_mode="queue") as tc:
        with (tc.tile_pool(name="wpool", bufs=1) as wp,
              tc.tile_pool(name="ctsp", bufs=B) as ctsp,
              tc.tile_pool(name="abp", bufs=B) as abp,
              tc.tile_pool(name="adjp", bufs=T) as adjp,
              tc.tile_pool(name="e1p", bufs=AHEAD + 4) as e1p,
              tc.tile_pool(name="e2p", bufs=6) as e2p,
              tc.tile_pool(name="scrp", bufs=6) as scrp,
              tc.tile_pool(name="finp", bufs=2) as finp,
              tc.tile_pool(name="psp", bufs=3, space="PSUM") as psp,
              tc.tile_pool(name="hps", bufs=2, space="PSUM") as hps):

            # ---- packed loads, one per HWDGE ring so they land in
            # parallel.  xpk (which gates the b=0 setup chain) is split so
            # its fixed+b0 slice arrives first; the scalar/vector queues are
            # idle this early so the descriptor-gen time there is free. ----
            xpkt = wp.tile([4, XPK_COLS], FP16, tag="xpk")
            xcut = XPK_FIX + XB
            nc.sync.dma_start(out=xpkt[:, 0:xcut], in_=xpk[:, 0:xcut])
            wpk = wp.tile([128, WPACK_COLS], F32, tag="wpk")
            nc.scalar.dma_start(out=wpk[:], in_=wpack[:])
            w2pkt = wp.tile([128, W2PK_COLS], FP16, tag="w2pk")
            nc.scalar.dma_start(out=w2pkt[:], in_=w2pk[:])
            we2nbd_s = wp.tile([128, 128], BFL, tag="we2nbd")
            nc.scalar.dma_start(out=we2nbd_s[:], in_=we2nbd[:])
            nc.scalar.dma_start(out=xpkt[:, xcut:XPK_COLS],
                                in_=xpk[:, xcut:XPK_COLS])

            def wslice(name):
                a, bb = _WP[name]
                return wpk[:, a:bb]
            b1s_s, b2s_s = wslice("b1s"), wslice("b2s")
            be2ns_s, bn2ns_s = wslice("be2ns"), wslice("bn2ns")
            bo1s_s = wslice("bo1s")
            bo2s_s = wpk[0:4, _WP["bo2s"][0]:_WP["bo2s"][1]]

            def w2slice(name, rows=128):
                a, bb = _W2[name]
                return w2pkt[0:rows, a:bb]
            w2bd_s = w2slice("w2bd")
            wn2nbd_s, wo1hbd_s = w2slice("wn2nbd"), w2slice("wo1hbd")
            wo2bd_s = w2slice("wo2bd")
            wjt2_s = xpkt[0:2, 0:128]
            wit_s = xpkt[0:2, 128:192]
            wo1xbd_s = xpkt[0:4, 192:XPK_FIX]

            def xslice(b, off, w, rows=2):
                a = XPK_FIX + b * XB + off
                return xpkt[0:rows, a:a + w]

            # fused-reduce accumulators: h1pre 8-sub-columns per (b, q);
            # two PSUM banks, one per batch-pair
            h1ps = [hps.tile([128, 2 * Q * 8], F32, tag="h1ps",
                             name=f"h1ps{i}")
                    for i in range(2)]
            h1v = [t[:].rearrange("p (b q e) -> p b q e", b=2, e=8)
                   for t in h1ps]

            # ---- per-batch setup: CTS (stacked C.T) and ABIAS (A + b1);
            # emitted lazily so batch 0 reaches the main loop ASAP ----
            CTS, AB = [None] * B, [None] * B

            def emit_setup(b):
                ps = psp.tile([128, 1024], F32, tag="ps")
                nc.tensor.matmul(ps[:, 0:512], lhsT=wjt2_s,
                                 rhs=xslice(b, 0, 512), start=True, stop=True)
                nc.tensor.matmul(ps[0:64, 512:512 + Q], lhsT=wit_s,
                                 rhs=xslice(b, 512, Q), start=True, stop=True)
                nc.tensor.matmul(ps[64:128, 512:512 + Q], lhsT=wit_s,
                                 rhs=xslice(b, 512 + Q, Q),
                                 start=True, stop=True)
                cts = ctsp.tile([128, N], FP16, tag="cts")
                nc.scalar.copy(cts[:], ps[:, 0:512])
                ab = abp.tile([128, Q], F32, tag="ab")
                nc.vector.tensor_scalar_add(out=ab[:],
                                            in0=ps[0:128, 512:512 + Q],
                                            scalar1=b1s_s)
                CTS[b], AB[b] = cts, ab

            for b in range(B):
                emit_setup(b)

            # ---- main loop, software pipelined ----
            ADJ = [None] * T
            E1 = [None] * G     # e1 tiles, built AHEAD iterations early
            E2 = [None] * G
            SCR = [None] * G

            def emit_e1(g):
                t, b = divmod(g, B)
                e1m = e1p.tile([128, 1024], FP16, tag="e1", name=f"e1_{g}")
                for k in range(2):
                    q = 2 * t + k
                    nc.vector.tensor_scalar(
                        out=e1m[:, 512 * k:512 * (k + 1)], in0=CTS[b][:],
                        scalar1=AB[b][:, q:q + 1], scalar2=0.0,
                        op0=AL.add, op1=AL.max)
                E1[g] = e1m

            def emit_tt(g):
                t, b = divmod(g, B)
                scrm = scrp.tile([128, 1024], BFL, tag="scr", name=f"scr{g}")
                nc.vector.tensor_tensor(
                    out=scrm[:].rearrange("p (u j) -> p u j", u=2),
                    in0=E2[g][:].rearrange("p (u j) -> p u j", u=2),
                    in1=ADJ[t][:].rearrange("p (u j) -> p u j", u=2),
                    op=AL.mult)
                SCR[g] = scrm

            def emit_red(g):
                t, b = divmod(g, B)
                for k in range(2):
                    q = 2 * t + k
                    sl = h1v[b // 2][:, b % 2, q:q + 1, :]       # [128, 1, 8]
                    nc.tensor.matmul(sl.broadcast_to((128, 64, 8)),
                                     lhsT=we2nbd_s[:],
                                     rhs=SCR[g][:, 512 * k:512 * (k + 1)],
                                     start=True, stop=True)

            for g in range(AHEAD):
                emit_e1(g)

            def load_adj(t):
                # t-major iteration means each adj tile serves 4 consecutive
                # iterations, so the sync ring alone keeps up and the ACT /
                # gpsimd queues carry no steady-state DMA work at all
                adjt = adjp.tile([128, 1024], BFL, tag="adj")
                for r in range(4):
                    nc.sync.dma_start(
                        out=adjt[64 * (r % 2):64 * (r % 2 + 1),
                                 512 * (r // 2):512 * (r // 2 + 1)],
                        in_=adjr[4 * t + r:4 * t + r + 1, :]
                        .partition_broadcast(64))
                ADJ[t] = adjt

            load_adj(0)
            load_adj(1)
            for g in range(G):
                t, b = divmod(g, B)
                if b == 0 and t + 2 < T:
                    load_adj(t + 2)   # prefetch two t-groups ahead
                if g + AHEAD < G:
                    emit_e1(g + AHEAD)
                psm = psp.tile([128, 1024], F32, tag="ps")
                nc.tensor.matmul(psm[:, 0:512], lhsT=w2bd_s[:],
                                 rhs=E1[g][:, 0:512], start=True, stop=True)
                nc.tensor.matmul(psm[:, 512:1024], lhsT=w2bd_s[:],
                                 rhs=E1[g][:, 512:1024], start=True,
                                 stop=True)
                e2m = e2p.tile([128, 1024], BFL, tag="e2", name=f"e2_{g}")
                nc.scalar.activation(e2m[:], psm[:], AF.Relu, bias=b2s_s)
                E2[g] = e2m
                if g >= 1:
                    emit_tt(g - 1)
                if g >= 2:
                    emit_red(g - 2)
            emit_tt(G - 1)
            emit_red(G - 2)
            emit_red(G - 1)

            # ---- batched output MLP over all 4 b's: [128, B*Q] tiles ----
            BQ = B * Q
            # h1pre = sum over the 8 PSUM sub-columns; cols are b-major
            h1pre = finp.tile([128, BQ], F32, tag="h1pre")
            for p in range(2):
                nc.vector.tensor_reduce(
                    out=h1pre[:, 64 * p:64 * (p + 1)]
                    .rearrange("p (b q) -> p b q", b=2),
                    in_=h1v[p], axis=mybir.AxisListType.X, op=AL.add)
            h1 = finp.tile([128, BQ], FP16, tag="h1")
            nc.scalar.activation(h1[:], h1pre[:], AF.Relu, bias=be2ns_s)

            ps2 = psp.tile([128, 1024], F32, tag="ps")
            nc.tensor.matmul(ps2[:, 0:BQ], lhsT=wn2nbd_s, rhs=h1[:],
                             start=True, stop=True)
            h2 = finp.tile([128, BQ], FP16, tag="h2")
            nc.scalar.activation(h2[:], ps2[:, 0:BQ], AF.Relu, bias=bn2ns_s)

            ps3 = psp.tile([128, 1024], F32, tag="ps")
            # cat(x, h2) @ W_o1.T as two accumulating matmuls per b block
            for b in range(B):
                nc.tensor.matmul(ps3[:, b * Q:(b + 1) * Q], lhsT=wo1hbd_s,
                                 rhs=h2[:, b * Q:(b + 1) * Q],
                                 start=True, stop=False)
                nc.tensor.matmul(ps3[:, b * Q:(b + 1) * Q], lhsT=wo1xbd_s,
                                 rhs=xslice(b, 512 + 2 * Q, Q, rows=4),
                                 start=False, stop=True)
            h3 = finp.tile([128, BQ], FP16, tag="h3")
            nc.scalar.activation(h3[:], ps3[:, 0:BQ], AF.Relu, bias=bo1s_s)

            ps4 = psp.tile([128, 1024], F32, tag="ps")
            nc.tensor.matmul(ps4[0:4, 0:BQ], lhsT=wo2bd_s, rhs=h3[:],
                             start=True, stop=True)
            outs = finp.tile([4, BQ], F32, tag="outs")
            nc.scalar.activation(outs[:], ps4[0:4, 0:BQ], AF.Identity,
                                 bias=bo2s_s)
            for b in range(B):
                eng = nc.sync if b % 2 == 0 else nc.scalar
                eng.dma_start(out=out_re[b], in_=outs[:, b * Q:(b + 1) * Q])

    nc.compile()
    return nc


def _get_nc():
    if "nc" not in _STATE:
        _STATE["nc"] = _build_nc()
    return _STATE["nc"]


def _prep_maps(inputs):
    import ml_dtypes
    bfl = ml_dtypes.bfloat16
    fp16 = np.float16
    f32 = np.float32

    x = np.ascontiguousarray(np.asarray(inputs["input"], f32))      # [B,N,D]
    adj = np.ascontiguousarray(np.asarray(inputs["adj"], f32))      # [N,N]
    W_n2e = np.asarray(inputs["W_n2e"], f32)   # [H, 2D]
    b_n2e = np.asarray(inputs["b_n2e"], f32)
    W_e2e = np.asarray(inputs["W_e2e"], f32)
    b_e2e = np.asarray(inputs["b_e2e"], f32)
    W_e2n = np.asarray(inputs["W_e2n"], f32)
    b_e2n = np.asarray(inputs["b_e2n"], f32)
    W_n2n = np.asarray(inputs["W_n2n"], f32)
    b_n2n = np.asarray(inputs["b_n2n"], f32)
    W_o1 = np.asarray(inputs["W_o1"], f32)     # [H, D+H]
    b_o1 = np.asarray(inputs["b_o1"], f32)
    W_o2 = np.asarray(inputs["W_o2"], f32)     # [O, H]
    b_o2 = np.asarray(inputs["b_o2"], f32)

    Wi, Wj = W_n2e[:, :D], W_n2e[:, D:]

    def bd(w):  # blockdiag(w, w)
        r, c = w.shape
        z = np.zeros((2 * r, 2 * c), f32)
        z[:r, :c] = w
        z[r:, c:] = w
        return z

    wpack = np.zeros((128, WPACK_COLS), f32)

    def put(name, val, rows=128):
        a, bb = _WP[name]
        wpack[:rows, a:bb] = val
    put("b1s", np.concatenate([b_n2e, b_n2e]).reshape(128, 1))
    put("b2s", np.concatenate([b_e2e, b_e2e]).reshape(128, 1))
    put("be2ns", np.concatenate([b_e2n, b_e2n]).reshape(128, 1))
    put("bn2ns", np.concatenate([b_n2n, b_n2n]).reshape(128, 1))
    put("bo1s", np.concatenate([b_o1, b_o1]).reshape(128, 1))
    put("bo2s", np.concatenate([b_o2, b_o2]).reshape(4, 1), rows=4)

    w2pack = np.zeros((128, W2PK_COLS), f32)

    def put2(name, val, rows=128):
        a, bb = _W2[name]
        w2pack[:rows, a:bb] = val
    put2("w2bd", bd(W_e2e.T))
    put2("wn2nbd", bd(W_n2n.T))
    put2("wo1hbd", bd(W_o1[:, D:].T))
    put2("wo2bd", bd(W_o2.T))

    maps = []
    for c in range(NCORES):
        sl = slice(c * IB, (c + 1) * IB)
        xc = x[:, sl]                                    # [B, IB, D]
        xpk = np.zeros((4, XPK_COLS), f32)
        xpk[0:2, 0:128] = np.concatenate([Wj.T, Wj.T], axis=1)
        xpk[0:2, 128:192] = Wi.T
        xpk[0:4, 192:XPK_FIX] = bd(W_o1[:, :D].T)
        for b in range(B):
            a = XPK_FIX + b * XB
            xpk[0:2, a:a + 512] = x[b].T
            xpk[0:2, a + 512:a + 512 + Q] = xc[b, 0::2].T
            xpk[0:2, a + 512 + Q:a + 512 + 2 * Q] = xc[b, 1::2].T
            xpk[0:4, a + 512 + 2 * Q:a + 512 + 3 * Q] = \
                xc[b].reshape(Q, 2 * D).T                # rows e*2+d
        m = {
            "wpack": wpack,
            "w2pk": w2pack.astype(fp16),
            "xpk": xpk.astype(fp16),
            "adjr": adj[sl].astype(bfl),
            "we2nbd": bd(W_e2n.T).astype(bfl),
        }
        maps.append({k: np.ascontiguousarray(v) for k, v in m.items()})
    return maps


def run(inputs, trace=False, **kw):
    from concourse.bass_utils import run_bass_kernel_spmd
    nc = _get_nc()
    maps = _prep_maps(inputs)
    res = run_bass_kernel_spmd(nc, maps, list(range(NCORES)), trace=trace, **kw)
    out = np.concatenate([res.results[c]["out"] for c in range(NCORES)], axis=1)
    return np.ascontiguousarray(out, dtype=np.float32), res


def kernel(**inputs):
    out, _ = run(inputs, trace=False)
    return out
